# revision 2
# baseline (speedup 1.0000x reference)
"""Expert-parallel grouped matmul (MoE BatchLinear) for 8 Trainium2 NeuronCores.

Problem: y[t] = x[t] @ W[g(t)] where tokens are grouped contiguously by expert
g (G=64 experts, counts given at runtime). Sharding: expert-parallel — core c
owns experts [8c, 8c+8) and the contiguous token rows routed to them. The
"all-to-all" is done host-side: kernel() receives full inputs, slices/pads
per-core token blocks, and scatters per-core outputs back.

Device kernel (SPMD, one program on 8 cores):
  for each local expert e (8 per core):
    xT_e resident in SBUF as [128ki, 8ko, Te] (host pre-transposed)
    for each 1024-wide n-quarter of W_e (slab [128ki, 8ko, 1024], dbl-buffered):
      for each 128-token m-tile:
        8 k-steps x 2 matmuls (fp32r, N=512) accumulate into 2 PSUM banks
        DVE copy PSUM -> SBUF staging, DMA staging -> y
fp32r streams at 1 cycle/row (vs 4 for fp32) with ~1e-4 relative rounding.
"""

import numpy as np

G, N_TOK, D_IN, D_OUT, CAP = 64, 32768, 1024, 4096, 768
M_CORES = 8
EPC = G // M_CORES          # experts per core
P = 128                     # partitions / k-tile / m-tile
KO = D_IN // P              # 8 k-tiles
NQW = 1024                  # n-quarter width (W slab columns)
NQ = D_OUT // NQW           # 4 quarters
NB = NQW // 512             # psum banks per quarter
USE_F32R = True             # False -> exact fp32 matmul at 1/4 rate

_cache = {}


def _build(mt):
    """Compile the SPMD program for per-expert-slot m-tile counts mt (len EPC)."""
    import concourse.mybir as mybir
    import concourse.tile as tile
    from concourse import bacc

    f32 = mybir.dt.float32
    fmm = mybir.dt.float32r if USE_F32R else f32
    tpad = P * sum(mt)

    nc = bacc.Bacc("TRN2", target_bir_lowering=False, debug=False)
    xT_d = nc.dram_tensor("xT", [KO, P, tpad], fmm, kind="ExternalInput")
    w_d = nc.dram_tensor("W", [EPC, KO, P, D_OUT], fmm, kind="ExternalInput")
    y_d = nc.dram_tensor("y", [tpad, D_OUT], f32, kind="ExternalOutput")
    xT, w_ap, y = xT_d.ap(), w_d.ap(), y_d.ap()

    with tile.TileContext(nc) as tc:
        with (
            tc.tile_pool(name="wq", bufs=2) as wq_pool,
            tc.tile_pool(name="xt", bufs=2) as xt_pool,
            tc.tile_pool(name="st", bufs=3) as st_pool,
            tc.tile_pool(name="ps", bufs=8, space="PSUM") as ps_pool,
        ):
            t0 = 0
            for e in range(EPC):
                if mt[e] == 0:
                    continue
                te = P * mt[e]
                xt = xt_pool.tile([P, KO, te], fmm, tag="xt")
                nc.sync.dma_start(
                    out=xt[:],
                    in_=xT[:, :, t0 : t0 + te].rearrange("ko ki t -> ki ko t"),
                )
                for q in range(NQ):
                    wq = wq_pool.tile([P, KO, NQW], fmm, tag="wq")
                    nc.sync.dma_start(
                        out=wq[:],
                        in_=w_ap[e, :, :, q * NQW : (q + 1) * NQW].rearrange(
                            "ko ki n -> ki ko n"
                        ),
                    )
                    for m in range(mt[e]):
                        pss = [ps_pool.tile([P, 512], f32, tag="ps", name="ps") for _ in range(NB)]
                        for k in range(KO):
                            lhsT = xt[:, k, m * P : (m + 1) * P]
                            for nn in range(NB):
                                nc.tensor.matmul(
                                    pss[nn][:],
                                    lhsT,
                                    wq[:, k, nn * 512 : (nn + 1) * 512],
                                    start=(k == 0),
                                    stop=(k == KO - 1),
                                )
                        st = st_pool.tile([P, NQW], f32, tag="st")
                        for nn in range(NB):
                            nc.vector.tensor_copy(
                                st[:, nn * 512 : (nn + 1) * 512], pss[nn][:]
                            )
                        nc.sync.dma_start(
                            out=y[t0 + m * P : t0 + (m + 1) * P, q * NQW : (q + 1) * NQW],
                            in_=st[:],
                        )
                t0 += te
    nc.compile()
    return nc, tpad


def _prepare(x, weight, counts):
    """Host-side all-to-all: per-core padded token blocks + weight slices."""
    starts = np.zeros(G + 1, np.int64)
    np.cumsum(counts, out=starts[1:])
    cnt = counts.reshape(M_CORES, EPC)
    mt = tuple(int(v) for v in np.ceil(cnt / P).astype(np.int64).max(axis=0))
    tpad = P * sum(mt)

    in_maps, metas = [], []
    for c in range(M_CORES):
        xc = np.zeros((tpad, D_IN), np.float32)
        off, meta = 0, []
        for j in range(EPC):
            g = c * EPC + j
            s, n = int(starts[g]), int(counts[g])
            n = min(n, N_TOK - s) if s < N_TOK else 0
            if n > 0:
                xc[off : off + n] = x[s : s + n]
            meta.append((off, s, n))
            off += P * mt[j]
        xTc = np.ascontiguousarray(xc.T).reshape(KO, P, tpad)
        wc = np.ascontiguousarray(weight[c * EPC : (c + 1) * EPC]).reshape(
            EPC, KO, P, D_OUT
        )
        in_maps.append({"xT": xTc, "W": wc})
        metas.append(meta)
    return mt, in_maps, metas


def _run(x, weight, counts, trace=False, trace_cores=None):
    from concourse.bass_utils import run_bass_kernel_spmd

    x = np.ascontiguousarray(np.asarray(x, dtype=np.float32))
    weight = np.ascontiguousarray(np.asarray(weight, dtype=np.float32))
    counts = np.asarray(counts).astype(np.int64)
    assert counts.shape == (G,)

    mt, in_maps, metas = _prepare(x, weight, counts)
    if mt not in _cache:
        _cache[mt] = _build(mt)
    nc, _tpad = _cache[mt]

    res = run_bass_kernel_spmd(
        nc,
        in_maps,
        core_ids=list(range(M_CORES)),
        trace=trace,
        trace_cores=trace_cores,
    )
    out = np.zeros((N_TOK, D_OUT), np.float32)
    for c in range(M_CORES):
        yc = res.results[c]["y"]
        for off, s, n in metas[c]:
            if n > 0:
                out[s : s + n] = yc[off : off + n]
    return out, res


def kernel(x, weight, num_inputs_per_group):
    out, _ = _run(x, weight, num_inputs_per_group)
    return out


# revision 3
# speedup vs baseline: 1.0875x; 1.0875x over previous
"""Expert-parallel grouped matmul (MoE BatchLinear) for 8 Trainium2 NeuronCores.

Problem: y[t] = x[t] @ W[g(t)] where tokens are grouped contiguously by expert
g (G=64 experts, counts given at runtime). Sharding: expert-parallel — core c
owns experts [8c, 8c+8) and the contiguous token rows routed to them. The
"all-to-all" is done host-side: kernel() receives full inputs, slices/pads
per-core token blocks, and scatters per-core outputs back.

Device kernel (SPMD, one program on 8 cores):
  for each local expert e (8 per core):
    xT_e resident in SBUF as [128ki, 8ko, Te] (host pre-transposed)
    for each 1024-wide n-quarter of W_e (slab [128ki, 8ko, 1024], triple-buf):
      for each 128-token m-tile:
        8 k-steps x 2 matmuls (fp32r, N=512) accumulate into 2 PSUM banks
        DVE copy PSUM -> SBUF staging, DMA staging -> y (scalar-engine ring)

All DRAM layouts are chosen so every DMA reads/writes fully-contiguous
per-partition runs: W as [e, q, ki, ko, n] (32KB/partition lines), xT as
per-expert [ki, ko, Te] blocks, y as [mtile, q, 128, 1024] (contiguous 512KB
blocks, reordered host-side).

fp32r streams at 1 cycle/row (vs 4 for fp32) with ~1.5e-4 relative rounding.
"""

import numpy as np

G, N_TOK, D_IN, D_OUT, CAP = 64, 32768, 1024, 4096, 768
M_CORES = 8
EPC = G // M_CORES          # experts per core
P = 128                     # partitions / k-tile / m-tile
KO = D_IN // P              # 8 k-tiles
NQW = 1024                  # n-slab width (W slab columns)
NQ = D_OUT // NQW           # 4 slabs
NB = NQW // 512             # psum banks per slab
USE_F32R = True             # False -> exact fp32 matmul at 1/4 rate

_cache = {}


def _build(mt):
    """Compile the SPMD program for per-expert-slot m-tile counts mt (len EPC)."""
    import concourse.mybir as mybir
    import concourse.tile as tile
    from concourse import bacc

    f32 = mybir.dt.float32
    fmm = mybir.dt.float32r if USE_F32R else f32
    n_mtiles = sum(mt)

    nc = bacc.Bacc("TRN2", target_bir_lowering=False, debug=False)
    xt_d = [
        nc.dram_tensor(f"xT{e}", [P, KO, P * mt[e]], fmm, kind="ExternalInput")
        for e in range(EPC)
    ]
    w_d = nc.dram_tensor("W", [EPC, NQ, P, KO, NQW], fmm, kind="ExternalInput")
    y_d = nc.dram_tensor("y", [n_mtiles, NQ, P, NQW], f32, kind="ExternalOutput")
    w_ap, y = w_d.ap(), y_d.ap()

    with tile.TileContext(nc) as tc:
        with (
            tc.tile_pool(name="wq", bufs=3) as wq_pool,
            tc.tile_pool(name="xt", bufs=2) as xt_pool,
            tc.tile_pool(name="st", bufs=4) as st_pool,
            tc.tile_pool(name="ps", bufs=8, space="PSUM") as ps_pool,
        ):
            mi0 = 0  # global m-tile index
            for e in range(EPC):
                if mt[e] == 0:
                    continue
                te = P * mt[e]
                xt = xt_pool.tile([P, KO, te], fmm, tag="xt")
                nc.sync.dma_start(out=xt[:], in_=xt_d[e].ap())
                for q in range(NQ):
                    wq = wq_pool.tile([P, KO, NQW], fmm, tag="wq")
                    nc.sync.dma_start(out=wq[:], in_=w_ap[e, q])
                    for m in range(mt[e]):
                        pss = [
                            ps_pool.tile([P, 512], f32, tag="ps", name="ps")
                            for _ in range(NB)
                        ]
                        for k in range(KO):
                            lhsT = xt[:, k, m * P : (m + 1) * P]
                            for nn in range(NB):
                                nc.tensor.matmul(
                                    pss[nn][:],
                                    lhsT,
                                    wq[:, k, nn * 512 : (nn + 1) * 512],
                                    start=(k == 0),
                                    stop=(k == KO - 1),
                                )
                        st = st_pool.tile([P, NQW], f32, tag="st")
                        for nn in range(NB):
                            nc.vector.tensor_copy(
                                st[:, nn * 512 : (nn + 1) * 512], pss[nn][:]
                            )
                        nc.scalar.dma_start(out=y[mi0 + m, q], in_=st[:])
                mi0 += mt[e]
    nc.compile()
    return nc


def _prepare(x, weight, counts):
    """Host-side all-to-all: per-core padded token blocks + weight slices."""
    starts = np.zeros(G + 1, np.int64)
    np.cumsum(counts, out=starts[1:])
    cnt = counts.reshape(M_CORES, EPC)
    mt = tuple(int(v) for v in np.ceil(cnt / P).astype(np.int64).max(axis=0))

    in_maps, metas = [], []
    for c in range(M_CORES):
        im = {}
        meta = []
        mi0 = 0
        for j in range(EPC):
            g = c * EPC + j
            s, n = int(starts[g]), int(counts[g])
            n = min(n, N_TOK - s) if s < N_TOK else 0
            te = P * mt[j]
            xe = np.zeros((te, D_IN), np.float32)
            if n > 0:
                xe[:n] = x[s : s + n]
            # [te, D_IN] -> [D_IN, te] -> [KO, P, te] -> [P, KO, te]
            im[f"xT{j}"] = np.ascontiguousarray(
                xe.T.reshape(KO, P, te).transpose(1, 0, 2)
            )
            meta.append((mi0, s, n))
            mi0 += mt[j]
        # weight [EPC, D_IN, D_OUT] -> [e, q, ki, ko, n]
        wc = weight[c * EPC : (c + 1) * EPC].reshape(EPC, KO, P, NQ, NQW)
        im["W"] = np.ascontiguousarray(wc.transpose(0, 3, 2, 1, 4))
        in_maps.append(im)
        metas.append(meta)
    return mt, in_maps, metas


def _run(x, weight, counts, trace=False, trace_cores=None):
    from concourse.bass_utils import run_bass_kernel_spmd

    x = np.ascontiguousarray(np.asarray(x, dtype=np.float32))
    weight = np.ascontiguousarray(np.asarray(weight, dtype=np.float32))
    counts = np.asarray(counts).astype(np.int64)
    assert counts.shape == (G,)

    mt, in_maps, metas = _prepare(x, weight, counts)
    if mt not in _cache:
        _cache[mt] = _build(mt)
    nc = _cache[mt]

    res = run_bass_kernel_spmd(
        nc,
        in_maps,
        core_ids=list(range(M_CORES)),
        trace=trace,
        trace_cores=trace_cores,
    )
    out = np.zeros((N_TOK, D_OUT), np.float32)
    for c in range(M_CORES):
        yc = res.results[c]["y"]  # [n_mtiles, NQ, P, NQW]
        n_mtiles = yc.shape[0]
        # -> [n_mtiles, P, NQ, NQW] -> [n_mtiles*P, D_OUT]
        yc = yc.transpose(0, 2, 1, 3).reshape(n_mtiles * P, D_OUT)
        for mi0, s, n in metas[c]:
            if n > 0:
                out[s : s + n] = yc[mi0 * P : mi0 * P + n]
    return out, res


def kernel(x, weight, num_inputs_per_group):
    out, _ = _run(x, weight, num_inputs_per_group)
    return out


# revision 4
# speedup vs baseline: 1.4748x; 1.3562x over previous
"""Expert-parallel grouped matmul (MoE BatchLinear) for 8 Trainium2 NeuronCores.

Problem: y[t] = x[t] @ W[g(t)] where tokens are grouped contiguously by expert
g (G=64 experts, counts given at runtime). Sharding: expert-parallel — core c
owns experts [8c, 8c+8) and the contiguous token rows routed to them. The
"all-to-all" is done host-side: kernel() receives full inputs, slices/pads
per-core token blocks, and scatters per-core outputs back.

Device kernel (SPMD, one program on 8 cores):
  for each local expert e (8 per core):
    xT_e resident in SBUF as [128ki, 8ko, Te] (host pre-transposed)
    for each NQW-wide n-slab of W_e (slab [128ki, 8ko, NQW], triple-buf):
      for each 128-token m-tile:
        8 k-steps x NB matmuls (N=512) accumulate into NB PSUM banks
        DVE copy PSUM -> SBUF staging, DMA staging -> y (scalar-engine ring)

All DRAM layouts are chosen so every DMA reads/writes fully-contiguous
per-partition runs: W as [e, q, ki, ko, n], xT as per-expert [ki, ko, Te]
blocks, y as [mtile, q, 128, NQW] contiguous blocks (reordered host-side).

Numerics: operands stream as fp16 (1 PE cycle/row, fp32 PSUM accumulation).
Measured absmax/scale error ~3e-4 vs the fp32 reference (fp32r: 1.5e-4 but
2x the input DMA traffic; plain fp32: exact but 4 cycles/row). MODE picks.
"""

import numpy as np

G, N_TOK, D_IN, D_OUT, CAP = 64, 32768, 1024, 4096, 768
M_CORES = 8
EPC = G // M_CORES          # experts per core
P = 128                     # partitions / k-tile / m-tile
KO = D_IN // P              # 8 k-tiles
MODE = "f16"                # "f16" | "f32r" | "f32"
NQW = 2048 if MODE == "f16" else 1024   # n-slab width (SBUF budget bound)
NQ = D_OUT // NQW
NB = NQW // 512             # psum banks per slab

_cache = {}


def _mm_dt(mybir):
    return {
        "f16": mybir.dt.float16,
        "f32r": mybir.dt.float32r,
        "f32": mybir.dt.float32,
    }[MODE]


def _np_dt():
    return np.float16 if MODE == "f16" else np.float32


def _build(mt):
    """Compile the SPMD program for per-expert-slot m-tile counts mt (len EPC)."""
    import concourse.mybir as mybir
    import concourse.tile as tile
    from concourse import bacc

    f32 = mybir.dt.float32
    fmm = _mm_dt(mybir)
    n_mtiles = sum(mt)

    nc = bacc.Bacc("TRN2", target_bir_lowering=False, debug=False)
    xt_d = [
        nc.dram_tensor(f"xT{e}", [P, KO, P * mt[e]], fmm, kind="ExternalInput")
        for e in range(EPC)
    ]
    w_d = nc.dram_tensor("W", [EPC, NQ, P, KO, NQW], fmm, kind="ExternalInput")
    y_d = nc.dram_tensor("y", [n_mtiles, NQ, P, NQW], f32, kind="ExternalOutput")
    w_ap, y = w_d.ap(), y_d.ap()

    with tile.TileContext(nc) as tc:
        with (
            tc.tile_pool(name="wq", bufs=3) as wq_pool,
            tc.tile_pool(name="xt", bufs=2) as xt_pool,
            tc.tile_pool(name="st", bufs=3) as st_pool,
            tc.tile_pool(name="ps", bufs=8, space="PSUM") as ps_pool,
        ):
            mi0 = 0  # global m-tile index
            for e in range(EPC):
                if mt[e] == 0:
                    continue
                te = P * mt[e]
                xt = xt_pool.tile([P, KO, te], fmm, tag="xt")
                nc.sync.dma_start(out=xt[:], in_=xt_d[e].ap())
                for q in range(NQ):
                    wq = wq_pool.tile([P, KO, NQW], fmm, tag="wq")
                    nc.sync.dma_start(out=wq[:], in_=w_ap[e, q])
                    for m in range(mt[e]):
                        pss = [
                            ps_pool.tile([P, 512], f32, tag="ps", name="ps")
                            for _ in range(NB)
                        ]
                        for k in range(KO):
                            lhsT = xt[:, k, m * P : (m + 1) * P]
                            for nn in range(NB):
                                nc.tensor.matmul(
                                    pss[nn][:],
                                    lhsT,
                                    wq[:, k, nn * 512 : (nn + 1) * 512],
                                    start=(k == 0),
                                    stop=(k == KO - 1),
                                )
                        st = st_pool.tile([P, NQW], f32, tag="st")
                        for nn in range(NB):
                            nc.vector.tensor_copy(
                                st[:, nn * 512 : (nn + 1) * 512], pss[nn][:]
                            )
                        nc.scalar.dma_start(out=y[mi0 + m, q], in_=st[:])
                mi0 += mt[e]
    nc.compile()
    return nc


def _prepare(x, weight, counts):
    """Host-side all-to-all: per-core padded token blocks + weight slices."""
    ndt = _np_dt()
    starts = np.zeros(G + 1, np.int64)
    np.cumsum(counts, out=starts[1:])
    cnt = counts.reshape(M_CORES, EPC)
    mt = tuple(int(v) for v in np.ceil(cnt / P).astype(np.int64).max(axis=0))

    in_maps, metas = [], []
    for c in range(M_CORES):
        im = {}
        meta = []
        mi0 = 0
        for j in range(EPC):
            g = c * EPC + j
            s, n = int(starts[g]), int(counts[g])
            n = min(n, N_TOK - s) if s < N_TOK else 0
            te = P * mt[j]
            xe = np.zeros((te, D_IN), ndt)
            if n > 0:
                xe[:n] = x[s : s + n]
            # [te, D_IN] -> [D_IN, te] -> [KO, P, te] -> [P, KO, te]
            im[f"xT{j}"] = np.ascontiguousarray(
                xe.T.reshape(KO, P, te).transpose(1, 0, 2)
            )
            meta.append((mi0, s, n))
            mi0 += mt[j]
        # weight [EPC, D_IN, D_OUT] -> [e, q, ki, ko, n]
        wc = weight[c * EPC : (c + 1) * EPC].reshape(EPC, KO, P, NQ, NQW)
        im["W"] = np.ascontiguousarray(wc.transpose(0, 3, 2, 1, 4).astype(ndt))
        in_maps.append(im)
        metas.append(meta)
    return mt, in_maps, metas


def _run(x, weight, counts, trace=False, trace_cores=None):
    from concourse.bass_utils import run_bass_kernel_spmd

    x = np.ascontiguousarray(np.asarray(x, dtype=np.float32))
    weight = np.ascontiguousarray(np.asarray(weight, dtype=np.float32))
    counts = np.asarray(counts).astype(np.int64)
    assert counts.shape == (G,)

    mt, in_maps, metas = _prepare(x, weight, counts)
    if mt not in _cache:
        _cache[mt] = _build(mt)
    nc = _cache[mt]

    res = run_bass_kernel_spmd(
        nc,
        in_maps,
        core_ids=list(range(M_CORES)),
        trace=trace,
        trace_cores=trace_cores,
    )
    out = np.zeros((N_TOK, D_OUT), np.float32)
    for c in range(M_CORES):
        yc = res.results[c]["y"]  # [n_mtiles, NQ, P, NQW]
        n_mtiles = yc.shape[0]
        # -> [n_mtiles, P, NQ, NQW] -> [n_mtiles*P, D_OUT]
        yc = yc.transpose(0, 2, 1, 3).reshape(n_mtiles * P, D_OUT)
        for mi0, s, n in metas[c]:
            if n > 0:
                out[s : s + n] = yc[mi0 * P : mi0 * P + n]
    return out, res


def kernel(x, weight, num_inputs_per_group):
    out, _ = _run(x, weight, num_inputs_per_group)
    return out


# revision 6
# speedup vs baseline: 1.5340x; 1.0402x over previous
"""Expert-parallel grouped matmul (MoE BatchLinear) for 8 Trainium2 NeuronCores.

Problem: y[t] = x[t] @ W[g(t)] where tokens are grouped contiguously by expert
g (G=64 experts, counts given at runtime). Sharding: expert-parallel — core c
owns experts [8c, 8c+8) and the contiguous token rows routed to them. The
"all-to-all" is done host-side: kernel() receives full inputs, slices/pads
per-core token blocks, and scatters per-core outputs back.

Device kernel (SPMD, one program on 8 cores):
  for each local expert e (8 per core):
    xT_e resident in SBUF as [128ki, 8ko, Te] (host pre-transposed)
    for each NQW-wide n-slab of W_e (slab [128ki, 8ko, NQW], triple-buf):
      for each 128-token m-tile:
        8 k-steps x NB matmuls (N=512) accumulate into NB PSUM banks
        DVE copy PSUM -> SBUF staging, DMA staging -> y (scalar-engine ring)

All DRAM layouts are chosen so every DMA reads/writes fully-contiguous
per-partition runs: W as [e, q, ki, ko, n], xT as per-expert [ki, ko, Te]
blocks, y as [mtile, q, 128, NQW] contiguous blocks (reordered host-side).

Numerics: operands stream as fp16 (1 PE cycle/row, fp32 PSUM accumulation).
Measured absmax/scale error ~3e-4 vs the fp32 reference (fp32r: 1.5e-4 but
2x the input DMA traffic; plain fp32: exact but 4 cycles/row). MODE picks.
"""

import numpy as np

G, N_TOK, D_IN, D_OUT, CAP = 64, 32768, 1024, 4096, 768
M_CORES = 8
EPC = G // M_CORES          # experts per core
P = 128                     # partitions / k-tile / m-tile
KO = D_IN // P              # 8 k-tiles
MODE = "f16"                # "f16" | "f32r" | "f32"
NQW = 2048 if MODE == "f16" else 1024   # n-slab width (SBUF budget bound)
NQ = D_OUT // NQW
NB = NQW // 512             # psum banks per slab

_cache = {}


def _mm_dt(mybir):
    return {
        "f16": mybir.dt.float16,
        "f32r": mybir.dt.float32r,
        "f32": mybir.dt.float32,
    }[MODE]


def _np_dt():
    return np.float16 if MODE == "f16" else np.float32


def _build(mt):
    """Compile the SPMD program for per-expert-slot m-tile counts mt (len EPC)."""
    import concourse.mybir as mybir
    import concourse.tile as tile
    from concourse import bacc

    f32 = mybir.dt.float32
    fmm = _mm_dt(mybir)
    n_mtiles = sum(mt)

    nc = bacc.Bacc("TRN2", target_bir_lowering=False, debug=False)
    xt_d = [
        nc.dram_tensor(f"xT{e}", [P, KO, P * mt[e]], fmm, kind="ExternalInput")
        for e in range(EPC)
    ]
    w_d = nc.dram_tensor("W", [EPC, NQ, P, KO, NQW], fmm, kind="ExternalInput")
    y_d = nc.dram_tensor("y", [n_mtiles, NQ, P, NQW], f32, kind="ExternalOutput")
    w_ap, y = w_d.ap(), y_d.ap()

    with tile.TileContext(nc) as tc:
        with (
            tc.tile_pool(name="wq", bufs=4) as wq_pool,
            tc.tile_pool(name="xt", bufs=2) as xt_pool,
            tc.tile_pool(name="st", bufs=3) as st_pool,
            tc.tile_pool(name="ps", bufs=8, space="PSUM") as ps_pool,
        ):
            mi0 = 0  # global m-tile index
            first = True
            for e in range(EPC):
                if mt[e] == 0:
                    continue
                te = P * mt[e]
                xt = xt_pool.tile([P, KO, te], fmm, tag="xt")
                nc.sync.dma_start(out=xt[:], in_=xt_d[e].ap())
                # ladder-size the very first expert's slabs so the first
                # matmul isn't gated on a full NQW-wide W transfer
                if first and NQW >= 2048:
                    widths = [512, 512, 1024] + [NQW] * ((D_OUT - NQW) // NQW)
                else:
                    widths = [NQW] * NQ
                first = False
                col = 0
                for wd in widths:
                    q, ncol, nb = col // NQW, col % NQW, wd // 512
                    wq = wq_pool.tile([P, KO, wd], fmm, tag="wq", name="wq")
                    nc.sync.dma_start(
                        out=wq[:], in_=w_ap[e, q, :, :, ncol : ncol + wd]
                    )
                    for m in range(mt[e]):
                        pss = [
                            ps_pool.tile([P, 512], f32, tag="ps", name="ps")
                            for _ in range(nb)
                        ]
                        for k in range(KO):
                            lhsT = xt[:, k, m * P : (m + 1) * P]
                            for nn in range(nb):
                                nc.tensor.matmul(
                                    pss[nn][:],
                                    lhsT,
                                    wq[:, k, nn * 512 : (nn + 1) * 512],
                                    start=(k == 0),
                                    stop=(k == KO - 1),
                                )
                        st = st_pool.tile([P, wd], f32, tag="st", name="st")
                        for nn in range(nb):
                            nc.vector.tensor_copy(
                                st[:, nn * 512 : (nn + 1) * 512], pss[nn][:]
                            )
                        nc.scalar.dma_start(
                            out=y[mi0 + m, q, :, ncol : ncol + wd], in_=st[:]
                        )
                    col += wd
                mi0 += mt[e]
    nc.compile()
    return nc


def _prepare(x, weight, counts):
    """Host-side all-to-all: per-core padded token blocks + weight slices."""
    ndt = _np_dt()
    starts = np.zeros(G + 1, np.int64)
    np.cumsum(counts, out=starts[1:])
    cnt = counts.reshape(M_CORES, EPC)
    mt = tuple(int(v) for v in np.ceil(cnt / P).astype(np.int64).max(axis=0))

    in_maps, metas = [], []
    for c in range(M_CORES):
        im = {}
        meta = []
        mi0 = 0
        for j in range(EPC):
            g = c * EPC + j
            s, n = int(starts[g]), int(counts[g])
            n = min(n, N_TOK - s) if s < N_TOK else 0
            te = P * mt[j]
            xe = np.zeros((te, D_IN), ndt)
            if n > 0:
                xe[:n] = x[s : s + n]
            # [te, D_IN] -> [D_IN, te] -> [KO, P, te] -> [P, KO, te]
            im[f"xT{j}"] = np.ascontiguousarray(
                xe.T.reshape(KO, P, te).transpose(1, 0, 2)
            )
            meta.append((mi0, s, n))
            mi0 += mt[j]
        # weight [EPC, D_IN, D_OUT] -> [e, q, ki, ko, n]
        wc = weight[c * EPC : (c + 1) * EPC].reshape(EPC, KO, P, NQ, NQW)
        im["W"] = np.ascontiguousarray(wc.transpose(0, 3, 2, 1, 4).astype(ndt))
        in_maps.append(im)
        metas.append(meta)
    return mt, in_maps, metas


def _run(x, weight, counts, trace=False, trace_cores=None):
    from concourse.bass_utils import run_bass_kernel_spmd

    x = np.ascontiguousarray(np.asarray(x, dtype=np.float32))
    weight = np.ascontiguousarray(np.asarray(weight, dtype=np.float32))
    counts = np.asarray(counts).astype(np.int64)
    assert counts.shape == (G,)

    mt, in_maps, metas = _prepare(x, weight, counts)
    if mt not in _cache:
        _cache[mt] = _build(mt)
    nc = _cache[mt]

    res = run_bass_kernel_spmd(
        nc,
        in_maps,
        core_ids=list(range(M_CORES)),
        trace=trace,
        trace_cores=trace_cores,
    )
    out = np.zeros((N_TOK, D_OUT), np.float32)
    for c in range(M_CORES):
        yc = res.results[c]["y"]  # [n_mtiles, NQ, P, NQW]
        n_mtiles = yc.shape[0]
        # -> [n_mtiles, P, NQ, NQW] -> [n_mtiles*P, D_OUT]
        yc = yc.transpose(0, 2, 1, 3).reshape(n_mtiles * P, D_OUT)
        for mi0, s, n in metas[c]:
            if n > 0:
                out[s : s + n] = yc[mi0 * P : mi0 * P + n]
    return out, res


def kernel(x, weight, num_inputs_per_group):
    out, _ = _run(x, weight, num_inputs_per_group)
    return out
